# revision 9
# baseline (speedup 1.0000x reference)
"""Trainium2 Bass kernel: multi-head self-attention (B=2, L=2048, D=1024, H=16).

Sharding: 8 NeuronCores = 2 batches x 4 head-groups (4 heads per core).
Each core computes, for its batch and its 4 heads:
  qkv projection -> full attention -> partial out-projection (its heads'
  contribution to out @ w_out).  The host sums the 4 head-group partials per
  batch and adds b_out.

Device dataflow (all layouts chosen so that no on-chip transpose is needed):
  - host passes x^T  [D, L]  (d-major), so d is on SBUF partitions.
  - qkT  = w_qk.T @ x^T      -> [c=512, L]   (Q^T / K^T per head, dk on partitions)
  - V    = x^T.T  @ w_v_aug  -> [L, 260]     (k-major V, plus a ones column per
                                              head that yields the softmax
                                              denominator for free)
  - S^T  = (K^T)ᵀ@ Q^T       -> [k, q] tiles (per head; 2 heads packed in the
                                              128-partition dim, contraction 64)
  - P^T  = exp(S^T)          (no max-subtraction: |scores| <= ~10 in f32, safe)
  - O^T_aug = V_augᵀ... psum += V_aug[k,65].T-contract -> [65, q]
              rows 0-63 = unnormalized head output (dv-major), row 64 = sum_k P
  - normalize: O^T = O^T_aug[0:64] * bcast(1/row64)
  - y    = O^T_cat.T @ w_out_local -> [L, 1024] partial, DMA'd out.

Matmuls run as float32r (TF32-like single-pass, full PE rate for moving dim
>= 256) via bitcast of f32 data; accumulation is f32 in PSUM.
"""

import sys

if "/opt/trn_rl_repo" not in sys.path:
    sys.path.insert(0, "/opt/trn_rl_repo")

import numpy as np

import concourse.bass as bass
import concourse.tile as tile
from concourse import mybir
from concourse.bass_utils import run_bass_kernel_spmd
from concourse.vector_clock import ScopedClock

B, L, D, H, DK = 2, 2048, 1024, 16, 64
HG = 4  # heads per core
F32 = mybir.dt.float32
F32R = mybir.dt.float32r
QC = 512  # l/q chunk width
NQ = L // QC  # 4 chunks
LT = L // 128  # 16 l tiles
KO = D // 128  # 8 contraction subtiles
CV = HG * (DK + 1)  # 260: v columns + per-head ones column

_PATCHED = False


def _patch_tile_drain():
    """This container's walrus rejects >1 sem wait on a ctrl instruction
    (setupSyncWait: 'Too many sync wait commands').  Tile's end-of-kernel
    drain accumulates one wait per outstanding semaphore; split the extras
    onto dedicated nops (same semantics: SP observes every sem before the
    final all-engine barrier)."""
    global _PATCHED
    if _PATCHED:
        return

    def _drain_and_barrier(self, tick_clock, wait_clock):
        nc = self.nc
        drain_inst = nc.sync.drain()
        wait_clock.add_sem_waits(
            drain_inst.ins, ScopedClock({None: tick_clock.global_clock})
        )
        si = drain_inst.ins.sync_info
        waits = list(si.on_wait or []) if si is not None else []
        if len(waits) > 1:
            si.on_wait = waits[:1]
            for w in waits[1:]:
                nop = nc.sync.nop()
                nsi = nop.ins.sync_info
                if nsi is None:
                    nop.ins.sync_info = mybir.SyncInfo(on_wait=[w], on_update=[])
                else:
                    nsi.on_wait = [w]
        nc.all_engine_barrier()
        popped = nc._tile_sem_poison_stack.pop()
        assert popped is self._sem_poison
        nc.clear_and_free_semaphores(list(self.sems.allocated().values()))
        nc.all_engine_barrier()

    tile.TileContext._drain_and_barrier = _drain_and_barrier
    _PATCHED = True


def _split_excess_waits(nc, max_waits=1):
    """This toolchain's walrus/ISA config allows only one sem wait per
    instruction, but Tile's wait assignment can attach several.  Hoist the
    extras onto same-engine nops immediately before the instruction (AND
    semantics preserved: the engine blocks on each in program order)."""
    for f in nc.m.functions:
        for blk in f.blocks:
            insts = list(blk.instructions)
            out = []
            changed = False
            for inst in insts:
                si = inst.sync_info
                waits = list(si.on_wait) if (si is not None and si.on_wait) else []
                if len(waits) > max_waits:
                    changed = True
                    for w in waits[:-max_waits]:
                        nop = mybir.InstNoOp(
                            name=f"I-wsplit-{nc.next_id()}",
                            engine=inst.engine,
                            ins=[],
                            outs=[],
                            sync_info=mybir.SyncInfo(on_wait=[w], on_update=[]),
                        )
                        nc.register_instruction(nop, overwrite=True)
                        out.append(nop)
                    si.on_wait = waits[-max_waits:]
                out.append(inst)
            if changed:
                blk.instructions = out


def build_nc():
    _patch_tile_drain()
    nc = bass.Bass()
    xT = nc.declare_dram_parameter("xT", [D, L], F32R, isOutput=False)
    wqk = nc.declare_dram_parameter("wqk", [D, 512], F32R, isOutput=False)
    bqk = nc.declare_dram_parameter("bqk", [512], F32, isOutput=False)
    wv = nc.declare_dram_parameter("wv", [D, CV], F32R, isOutput=False)
    bv = nc.declare_dram_parameter("bv", [CV], F32R, isOutput=False)
    wout = nc.declare_dram_parameter("wout", [2 * 128, 1024], F32R, isOutput=False)
    ones = nc.declare_dram_parameter("ones", [1, L], F32R, isOutput=False)
    y = nc.declare_dram_parameter("out", [L, D], F32, isOutput=True)

    Ident = mybir.ActivationFunctionType.Identity
    Exp = mybir.ActivationFunctionType.Exp

    with tile.TileContext(nc) as tc:
        with tc.tile_pool(name="per", bufs=1) as per:
            wqk_sb = per.tile([128, KO, 512], F32R)
            wv_sb = per.tile([128, KO, CV], F32R)
            wout_sb = per.tile([128, 2, 1024], F32R)
            bqk_sb = per.tile([128, 4], F32)
            bv_sb = per.tile([1, CV], F32R)
            ones_sb = per.tile([1, L], F32R)
            qkT_sb = per.tile([128, 4, L], F32R)
            v_sb = per.tile([128, LT, CV], F32R)
            oT_sb = per.tile([128, 2, L], F32R)

            for o in range(KO):
                nc.sync.dma_start(out=wqk_sb[:, o, :], in_=wqk[o * 128 : (o + 1) * 128, :])
                nc.sync.dma_start(out=wv_sb[:, o, :], in_=wv[o * 128 : (o + 1) * 128, :])
            nc.sync.dma_start(out=wout_sb[:, 0, :], in_=wout[0:128, :])
            nc.sync.dma_start(out=wout_sb[:, 1, :], in_=wout[128:256, :])
            nc.sync.dma_start(out=bqk_sb[:], in_=bqk.rearrange("(s p) -> p s", p=128))
            nc.sync.dma_start(out=bv_sb[:], in_=bv[None, :])
            nc.sync.dma_start(out=ones_sb[:], in_=ones[:])

            with (
                tc.tile_pool(name="xtp", bufs=1) as xtp,
                tc.tile_pool(name="psB", bufs=2, space="PSUM") as psB,
            ):
                xT_sb = xtp.tile([128, KO, L], F32R)
                for o in range(KO):
                    nc.sync.dma_start(
                        out=xT_sb[:, o, :], in_=xT[o * 128 : (o + 1) * 128, :]
                    )
                # qkT = w_qk.T @ x^T ; bias folded into the PSUM->SBUF copyback
                for s in range(4):
                    for n in range(NQ):
                        ps = psB.tile([128, QC], F32, tag="psqk")
                        for o in range(KO):
                            nc.tensor.matmul(
                                ps[:],
                                wqk_sb[:, o, s * 128 : (s + 1) * 128],
                                xT_sb[:, o, n * QC : (n + 1) * QC],
                                start=(o == 0),
                                stop=(o == KO - 1),
                            )
                        nc.scalar.activation(
                            qkT_sb[:, s, n * QC : (n + 1) * QC],
                            ps[:],
                            Ident,
                            bias=bqk_sb[:, s : s + 1],
                            scale=1.0,
                        )
                # V_aug = x @ w_v_aug (+ K=1 matmul adding bias and ones col)
                for lt in range(LT):
                    ps = psB.tile([128, CV], F32, tag="psv")
                    for o in range(KO):
                        nc.tensor.matmul(
                            ps[:],
                            xT_sb[:, o, lt * 128 : (lt + 1) * 128],
                            wv_sb[:, o, :],
                            start=(o == 0),
                            stop=False,
                        )
                    nc.tensor.matmul(
                        ps[:],
                        ones_sb[0:1, 0:128],
                        bv_sb[0:1, :],
                        start=False,
                        stop=True,
                    )
                    nc.vector.tensor_copy(out=v_sb[:, lt, :], in_=ps[:])

            # attention: per head pair (even head on partitions 0-63, odd on
            # 64-127 -> two row-group-packed K=64 matmuls run concurrently)
            with (
                tc.tile_pool(name="pt", bufs=3) as ptp,
                tc.tile_pool(name="rcp", bufs=2) as rcp,
                tc.tile_pool(name="rdp", bufs=2, space="DRAM") as rdp,
                tc.tile_pool(name="psST", bufs=2, space="PSUM") as psST,
                tc.tile_pool(name="psOT", bufs=2, space="PSUM") as psOT,
            ):
                for p2 in range(2):
                    sq = 2 * p2  # q slot in qkT_sb
                    sk = 2 * p2 + 1  # k slot
                    for qc in range(NQ):
                        po_e = psOT.tile([65, QC], F32, tag="ote")
                        po_o = psOT.tile([65, QC], F32, tag="oto")
                        for kt in range(LT):
                            pse = psST.tile([128, QC], F32, tag="ste")
                            pso = psST.tile([128, QC], F32, tag="sto")
                            nc.tensor.matmul(
                                pse[:],
                                qkT_sb[0:64, sk, kt * 128 : (kt + 1) * 128],
                                qkT_sb[0:64, sq, qc * QC : (qc + 1) * QC],
                                start=True,
                                stop=True,
                            )
                            nc.tensor.matmul(
                                pso[:],
                                qkT_sb[64:128, sk, kt * 128 : (kt + 1) * 128],
                                qkT_sb[64:128, sq, qc * QC : (qc + 1) * QC],
                                start=True,
                                stop=True,
                            )
                            pte = ptp.tile([128, QC], F32R, tag="pte")
                            pto = ptp.tile([128, QC], F32R, tag="pto")
                            nc.scalar.activation(pte[:], pse[:], Exp)
                            nc.scalar.activation(pto[:], pso[:], Exp)
                            nc.tensor.matmul(
                                po_e[:],
                                v_sb[:, kt, (2 * p2) * 65 : (2 * p2) * 65 + 65],
                                pte[:],
                                start=(kt == 0),
                                stop=(kt == LT - 1),
                            )
                            nc.tensor.matmul(
                                po_o[:],
                                v_sb[:, kt, (2 * p2 + 1) * 65 : (2 * p2 + 1) * 65 + 65],
                                pto[:],
                                start=(kt == 0),
                                stop=(kt == LT - 1),
                            )
                        for he, po in ((0, po_e), (1, po_o)):
                            rc = rcp.tile([1, QC], F32, tag="rc")
                            rb = rcp.tile([64, QC], F32, tag="rb")
                            nc.vector.reciprocal(out=rc[:], in_=po[64:65, :])
                            rd = rdp.tile([1, QC], F32, tag="rd")
                            nc.sync.dma_start(out=rd[:], in_=rc[:])
                            nc.sync.dma_start(
                                out=rb[:], in_=rd[0:1, :].to_broadcast([64, QC])
                            )
                            nc.vector.tensor_mul(
                                out=oT_sb[he * 64 : (he + 1) * 64, p2, qc * QC : (qc + 1) * QC],
                                in0=po[0:64, :],
                                in1=rb[:],
                            )

            # partial out-projection: y = O^T_cat.T @ w_out_local
            with (
                tc.tile_pool(name="ysb", bufs=3) as ysb,
                tc.tile_pool(name="psE", bufs=2, space="PSUM") as psE,
            ):
                for lt in range(LT):
                    for n2 in range(2):
                        ps = psE.tile([128, QC], F32, tag="psy")
                        nc.tensor.matmul(
                            ps[:],
                            oT_sb[:, 0, lt * 128 : (lt + 1) * 128],
                            wout_sb[:, 0, n2 * QC : (n2 + 1) * QC],
                            start=True,
                            stop=False,
                        )
                        nc.tensor.matmul(
                            ps[:],
                            oT_sb[:, 1, lt * 128 : (lt + 1) * 128],
                            wout_sb[:, 1, n2 * QC : (n2 + 1) * QC],
                            start=False,
                            stop=True,
                        )
                        yt = ysb.tile([128, QC], F32, tag="yt")
                        nc.vector.tensor_copy(out=yt[:], in_=ps[:])
                        nc.sync.dma_start(
                            out=y[lt * 128 : (lt + 1) * 128, n2 * QC : (n2 + 1) * QC],
                            in_=yt[:],
                        )
    _split_excess_waits(nc)
    return nc


def make_in_maps(x, w_qkv, b_qkv, w_out):
    """Per-core input shards.  Core i: batch i//4, head group i%4 (4 heads).

    w_qk column order per core: slots of 128 = (pair0 q | pair0 k | pair1 q |
    pair1 k), each slot = [even head (64) | odd head (64)].  The 1/sqrt(dk)
    scale is folded into the q columns (and q bias entries).
    """
    in_maps = []
    for core in range(8):
        b, g = divmod(core, 4)
        heads = [4 * g + j for j in range(HG)]
        xT = np.ascontiguousarray(x[b].T)
        cols, bias = [], []
        for pair in range(2):
            for qk in range(2):
                for j in range(2):
                    h = heads[2 * pair + j]
                    base = h * 3 * DK + qk * DK
                    c = w_qkv[:, base : base + DK]
                    bb = b_qkv[base : base + DK]
                    if qk == 0:
                        c = c * (1.0 / np.sqrt(DK))
                        bb = bb * (1.0 / np.sqrt(DK))
                    cols.append(c)
                    bias.append(bb)
        wqk = np.ascontiguousarray(np.concatenate(cols, axis=1), dtype=np.float32)
        bqk = np.concatenate(bias).astype(np.float32)
        wv = np.zeros((D, CV), np.float32)
        bv = np.zeros((CV,), np.float32)
        for j, h in enumerate(heads):
            base = h * 3 * DK + 2 * DK
            wv[:, 65 * j : 65 * j + 64] = w_qkv[:, base : base + DK]
            bv[65 * j : 65 * j + 64] = b_qkv[base : base + DK]
            bv[65 * j + 64] = 1.0
        wo = np.ascontiguousarray(w_out[g * 256 : (g + 1) * 256, :], dtype=np.float32)
        in_maps.append(
            {
                "xT": xT,
                "wqk": wqk,
                "bqk": bqk,
                "wv": wv,
                "bv": bv,
                "wout": wo,
                "ones": np.ones((1, L), np.float32),
            }
        )
    return in_maps


def kernel(**inputs):
    x = np.asarray(inputs["x"], np.float32)
    w_qkv = np.asarray(inputs["w_qkv"], np.float32)
    b_qkv = np.asarray(inputs["b_qkv"], np.float32)
    w_out = np.asarray(inputs["w_out"], np.float32)
    b_out = np.asarray(inputs["b_out"], np.float32)

    in_maps = make_in_maps(x, w_qkv, b_qkv, w_out)
    nc = build_nc()
    res = run_bass_kernel_spmd(nc, in_maps, core_ids=list(range(8)))
    kernel.last_results = res

    out = np.zeros((B, L, D), np.float32)
    for core in range(8):
        out[core // 4] += res.results[core]["out"]
    out += b_out[None, None, :]
    return out


kernel.last_results = None


# revision 12
# speedup vs baseline: 1.0164x; 1.0164x over previous
"""Trainium2 Bass kernel: multi-head self-attention (B=2, L=2048, D=1024, H=16).

Sharding: 8 NeuronCores = 2 batches x 4 head-groups (4 heads per core).
Each core computes, for its batch and its 4 heads:
  qkv projection -> full attention -> partial out-projection (its heads'
  contribution to out @ w_out).  The host sums the 4 head-group partials per
  batch and adds b_out.

Device dataflow (all layouts chosen so that no on-chip transpose is needed):
  - host passes x^T  [D, L]  (d-major), so d is on SBUF partitions.
  - qkT  = w_qk.T @ x^T      -> [c=512, L]   (Q^T / K^T per head, dk on partitions)
  - V    = x^T.T  @ w_v_aug  -> [L, 260]     (k-major V, plus a ones column per
                                              head that yields the softmax
                                              denominator for free)
  - S^T  = (K^T)ᵀ@ Q^T       -> [k, q] tiles (per head; 2 heads packed in the
                                              128-partition dim, contraction 64)
  - P^T  = exp(S^T)          (no max-subtraction: |scores| <= ~10 in f32, safe)
  - O^T_aug = V_augᵀ... psum += V_aug[k,65].T-contract -> [65, q]
              rows 0-63 = unnormalized head output (dv-major), row 64 = sum_k P
  - normalize: O^T = O^T_aug[0:64] * bcast(1/row64)
  - y    = O^T_cat.T @ w_out_local -> [L, 1024] partial, DMA'd out.

Matmuls run as float32r (TF32-like single-pass, full PE rate for moving dim
>= 256) via bitcast of f32 data; accumulation is f32 in PSUM.
"""

import sys

if "/opt/trn_rl_repo" not in sys.path:
    sys.path.insert(0, "/opt/trn_rl_repo")

import numpy as np

import concourse.bass as bass
import concourse.tile as tile
from concourse import mybir
from concourse.bass_utils import run_bass_kernel_spmd
from concourse.vector_clock import ScopedClock

B, L, D, H, DK = 2, 2048, 1024, 16, 64
HG = 4  # heads per core
F32 = mybir.dt.float32
F32R = mybir.dt.float32r
QC = 512  # l/q chunk width
NQ = L // QC  # 4 chunks
LT = L // 128  # 16 l tiles
KO = D // 128  # 8 contraction subtiles
CV = HG * (DK + 1)  # 260: v columns + per-head ones column

_PATCHED = False


def _patch_tile_drain():
    """This container's walrus rejects >1 sem wait on a ctrl instruction
    (setupSyncWait: 'Too many sync wait commands').  Tile's end-of-kernel
    drain accumulates one wait per outstanding semaphore; split the extras
    onto dedicated nops (same semantics: SP observes every sem before the
    final all-engine barrier)."""
    global _PATCHED
    if _PATCHED:
        return

    def _drain_and_barrier(self, tick_clock, wait_clock):
        nc = self.nc
        drain_inst = nc.sync.drain()
        wait_clock.add_sem_waits(
            drain_inst.ins, ScopedClock({None: tick_clock.global_clock})
        )
        si = drain_inst.ins.sync_info
        waits = list(si.on_wait or []) if si is not None else []
        if len(waits) > 1:
            si.on_wait = waits[:1]
            for w in waits[1:]:
                nop = nc.sync.nop()
                nsi = nop.ins.sync_info
                if nsi is None:
                    nop.ins.sync_info = mybir.SyncInfo(on_wait=[w], on_update=[])
                else:
                    nsi.on_wait = [w]
        nc.all_engine_barrier()
        popped = nc._tile_sem_poison_stack.pop()
        assert popped is self._sem_poison
        nc.clear_and_free_semaphores(list(self.sems.allocated().values()))
        nc.all_engine_barrier()

    tile.TileContext._drain_and_barrier = _drain_and_barrier
    _PATCHED = True


def _split_excess_waits(nc, max_waits=1):
    """This toolchain's walrus/ISA config allows only one sem wait per
    instruction, but Tile's wait assignment can attach several.  Hoist the
    extras onto same-engine nops immediately before the instruction (AND
    semantics preserved: the engine blocks on each in program order)."""
    for f in nc.m.functions:
        for blk in f.blocks:
            insts = list(blk.instructions)
            out = []
            changed = False
            for inst in insts:
                si = inst.sync_info
                waits = list(si.on_wait) if (si is not None and si.on_wait) else []
                if len(waits) > max_waits:
                    changed = True
                    for w in waits[:-max_waits]:
                        nop = mybir.InstNoOp(
                            name=f"I-wsplit-{nc.next_id()}",
                            engine=inst.engine,
                            ins=[],
                            outs=[],
                            sync_info=mybir.SyncInfo(on_wait=[w], on_update=[]),
                        )
                        nc.register_instruction(nop, overwrite=True)
                        out.append(nop)
                    si.on_wait = waits[-max_waits:]
                out.append(inst)
            if changed:
                blk.instructions = out


def build_nc():
    _patch_tile_drain()
    nc = bass.Bass()
    xT = nc.declare_dram_parameter("xT", [D, L], F32R, isOutput=False)
    wqk = nc.declare_dram_parameter("wqk", [D, 512], F32R, isOutput=False)
    bqk = nc.declare_dram_parameter("bqk", [512], F32, isOutput=False)
    wv = nc.declare_dram_parameter("wv", [D, CV], F32R, isOutput=False)
    bv = nc.declare_dram_parameter("bv", [CV], F32R, isOutput=False)
    wout = nc.declare_dram_parameter("wout", [2 * 128, 1024], F32R, isOutput=False)
    ones = nc.declare_dram_parameter("ones", [1, L], F32R, isOutput=False)
    y = nc.declare_dram_parameter("out", [L, D], F32, isOutput=True)

    Ident = mybir.ActivationFunctionType.Identity
    Exp = mybir.ActivationFunctionType.Exp

    with tile.TileContext(nc) as tc:
        with tc.tile_pool(name="per", bufs=1) as per:
            wqk_sb = per.tile([128, KO, 512], F32R)
            wv_sb = per.tile([128, KO, CV], F32R)
            wout_sb = per.tile([128, 2, 1024], F32R)
            bqk_sb = per.tile([128, 4], F32)
            bv_sb = per.tile([1, CV], F32R)
            ones_sb = per.tile([1, L], F32R)
            qkT_sb = per.tile([128, 4, L], F32R)
            v_sb = per.tile([128, LT, CV], F32R)
            oT_sb = per.tile([128, 2, L], F32R)

            for o in range(KO):
                nc.sync.dma_start(out=wqk_sb[:, o, :], in_=wqk[o * 128 : (o + 1) * 128, :])
                nc.sync.dma_start(out=wv_sb[:, o, :], in_=wv[o * 128 : (o + 1) * 128, :])
            nc.sync.dma_start(out=wout_sb[:, 0, :], in_=wout[0:128, :])
            nc.sync.dma_start(out=wout_sb[:, 1, :], in_=wout[128:256, :])
            nc.sync.dma_start(out=bqk_sb[:], in_=bqk.rearrange("(s p) -> p s", p=128))
            nc.sync.dma_start(out=bv_sb[:], in_=bv[None, :])
            nc.sync.dma_start(out=ones_sb[:], in_=ones[:])

            with (
                tc.tile_pool(name="xtp", bufs=1) as xtp,
                tc.tile_pool(name="psB", bufs=1, space="PSUM") as psB,
                tc.tile_pool(name="psC", bufs=2, space="PSUM") as psC,
            ):
                xT_sb = xtp.tile([128, KO, L], F32R)
                for o in range(KO):
                    nc.sync.dma_start(
                        out=xT_sb[:, o, :], in_=xT[o * 128 : (o + 1) * 128, :]
                    )
                # qkT = w_qk.T @ x^T ; bias folded into the PSUM->SBUF copyback.
                # o (the d contraction) is the middle loop so the first matmuls
                # only need the first x^T chunk - PE starts while x^T streams in.
                for s in range(4):
                    pss = [
                        psB.tile([128, QC], F32, tag=f"psqk{n}", name=f"psqk{n}")
                        for n in range(NQ)
                    ]
                    for o in range(KO):
                        for n in range(NQ):
                            nc.tensor.matmul(
                                pss[n][:],
                                wqk_sb[:, o, s * 128 : (s + 1) * 128],
                                xT_sb[:, o, n * QC : (n + 1) * QC],
                                start=(o == 0),
                                stop=(o == KO - 1),
                            )
                    for n in range(NQ):
                        nc.scalar.activation(
                            qkT_sb[:, s, n * QC : (n + 1) * QC],
                            pss[n][:],
                            Ident,
                            bias=bqk_sb[:, s : s + 1],
                            scale=1.0,
                        )
                # V_aug = x @ w_v_aug (+ K=1 matmul adding bias and ones col)
                for lt in range(LT):
                    ps = psC.tile([128, CV], F32, tag="psv")
                    for o in range(KO):
                        nc.tensor.matmul(
                            ps[:],
                            xT_sb[:, o, lt * 128 : (lt + 1) * 128],
                            wv_sb[:, o, :],
                            start=(o == 0),
                            stop=False,
                        )
                    nc.tensor.matmul(
                        ps[:],
                        ones_sb[0:1, 0:128],
                        bv_sb[0:1, :],
                        start=False,
                        stop=True,
                    )
                    nc.vector.tensor_copy(out=v_sb[:, lt, :], in_=ps[:])

            # attention: per head pair (even head on partitions 0-63, odd on
            # 64-127 -> two row-group-packed K=64 matmuls run concurrently)
            with (
                tc.tile_pool(name="pt", bufs=3) as ptp,
                tc.tile_pool(name="rcp", bufs=2) as rcp,
                tc.tile_pool(name="rdp", bufs=2, space="DRAM") as rdp,
                tc.tile_pool(name="psST", bufs=2, space="PSUM") as psST,
                tc.tile_pool(name="psOT", bufs=2, space="PSUM") as psOT,
            ):
                def st_pair(sq, sk, qc, kt):
                    """S^T for both heads of the pair: two K=64 matmuls packed
                    into row groups 0-63 / 64-127 (concurrent on the PE)."""
                    pse = psST.tile([128, QC], F32, tag="ste")
                    pso = psST.tile([128, QC], F32, tag="sto")
                    nc.tensor.matmul(
                        pse[:],
                        qkT_sb[0:64, sk, kt * 128 : (kt + 1) * 128],
                        qkT_sb[0:64, sq, qc * QC : (qc + 1) * QC],
                        start=True,
                        stop=True,
                    )
                    nc.tensor.matmul(
                        pso[:],
                        qkT_sb[64:128, sk, kt * 128 : (kt + 1) * 128],
                        qkT_sb[64:128, sq, qc * QC : (qc + 1) * QC],
                        start=True,
                        stop=True,
                    )
                    return pse, pso

                for p2 in range(2):
                    sq = 2 * p2  # q slot in qkT_sb
                    sk = 2 * p2 + 1  # k slot
                    for qc in range(NQ):
                        po_e = psOT.tile([65, QC], F32, tag="ote")
                        po_o = psOT.tile([65, QC], F32, tag="oto")
                        # software pipeline: S^T runs one kt ahead of exp and
                        # the O^T accumulation so the PE never waits on ACT.
                        sts = st_pair(sq, sk, qc, 0)
                        for kt in range(LT):
                            pse, pso = sts
                            pte = ptp.tile([128, QC], F32R, tag="pte")
                            pto = ptp.tile([128, QC], F32R, tag="pto")
                            nc.scalar.activation(pte[:], pse[:], Exp)
                            nc.scalar.activation(pto[:], pso[:], Exp)
                            if kt + 1 < LT:
                                sts = st_pair(sq, sk, qc, kt + 1)
                            nc.tensor.matmul(
                                po_e[:],
                                v_sb[:, kt, (2 * p2) * 65 : (2 * p2) * 65 + 65],
                                pte[:],
                                start=(kt == 0),
                                stop=(kt == LT - 1),
                            )
                            nc.tensor.matmul(
                                po_o[:],
                                v_sb[:, kt, (2 * p2 + 1) * 65 : (2 * p2 + 1) * 65 + 65],
                                pto[:],
                                start=(kt == 0),
                                stop=(kt == LT - 1),
                            )
                        for he, po in ((0, po_e), (1, po_o)):
                            rc = rcp.tile([1, QC], F32, tag="rc")
                            rb = rcp.tile([64, QC], F32, tag="rb")
                            nc.vector.reciprocal(out=rc[:], in_=po[64:65, :])
                            rd = rdp.tile([1, QC], F32, tag="rd")
                            nc.sync.dma_start(out=rd[:], in_=rc[:])
                            nc.sync.dma_start(
                                out=rb[:], in_=rd[0:1, :].to_broadcast([64, QC])
                            )
                            nc.vector.tensor_mul(
                                out=oT_sb[he * 64 : (he + 1) * 64, p2, qc * QC : (qc + 1) * QC],
                                in0=po[0:64, :],
                                in1=rb[:],
                            )

            # partial out-projection: y = O^T_cat.T @ w_out_local
            with (
                tc.tile_pool(name="ysb", bufs=3) as ysb,
                tc.tile_pool(name="psE", bufs=2, space="PSUM") as psE,
            ):
                for lt in range(LT):
                    for n2 in range(2):
                        ps = psE.tile([128, QC], F32, tag="psy")
                        nc.tensor.matmul(
                            ps[:],
                            oT_sb[:, 0, lt * 128 : (lt + 1) * 128],
                            wout_sb[:, 0, n2 * QC : (n2 + 1) * QC],
                            start=True,
                            stop=False,
                        )
                        nc.tensor.matmul(
                            ps[:],
                            oT_sb[:, 1, lt * 128 : (lt + 1) * 128],
                            wout_sb[:, 1, n2 * QC : (n2 + 1) * QC],
                            start=False,
                            stop=True,
                        )
                        yt = ysb.tile([128, QC], F32, tag="yt")
                        nc.vector.tensor_copy(out=yt[:], in_=ps[:])
                        nc.sync.dma_start(
                            out=y[lt * 128 : (lt + 1) * 128, n2 * QC : (n2 + 1) * QC],
                            in_=yt[:],
                        )
    _split_excess_waits(nc)
    return nc


def make_in_maps(x, w_qkv, b_qkv, w_out):
    """Per-core input shards.  Core i: batch i//4, head group i%4 (4 heads).

    w_qk column order per core: slots of 128 = (pair0 q | pair0 k | pair1 q |
    pair1 k), each slot = [even head (64) | odd head (64)].  The 1/sqrt(dk)
    scale is folded into the q columns (and q bias entries).
    """
    in_maps = []
    for core in range(8):
        b, g = divmod(core, 4)
        heads = [4 * g + j for j in range(HG)]
        xT = np.ascontiguousarray(x[b].T)
        cols, bias = [], []
        for pair in range(2):
            for qk in range(2):
                for j in range(2):
                    h = heads[2 * pair + j]
                    base = h * 3 * DK + qk * DK
                    c = w_qkv[:, base : base + DK]
                    bb = b_qkv[base : base + DK]
                    if qk == 0:
                        c = c * (1.0 / np.sqrt(DK))
                        bb = bb * (1.0 / np.sqrt(DK))
                    cols.append(c)
                    bias.append(bb)
        wqk = np.ascontiguousarray(np.concatenate(cols, axis=1), dtype=np.float32)
        bqk = np.concatenate(bias).astype(np.float32)
        wv = np.zeros((D, CV), np.float32)
        bv = np.zeros((CV,), np.float32)
        for j, h in enumerate(heads):
            base = h * 3 * DK + 2 * DK
            wv[:, 65 * j : 65 * j + 64] = w_qkv[:, base : base + DK]
            bv[65 * j : 65 * j + 64] = b_qkv[base : base + DK]
            bv[65 * j + 64] = 1.0
        wo = np.ascontiguousarray(w_out[g * 256 : (g + 1) * 256, :], dtype=np.float32)
        in_maps.append(
            {
                "xT": xT,
                "wqk": wqk,
                "bqk": bqk,
                "wv": wv,
                "bv": bv,
                "wout": wo,
                "ones": np.ones((1, L), np.float32),
            }
        )
    return in_maps


def kernel(**inputs):
    x = np.asarray(inputs["x"], np.float32)
    w_qkv = np.asarray(inputs["w_qkv"], np.float32)
    b_qkv = np.asarray(inputs["b_qkv"], np.float32)
    w_out = np.asarray(inputs["w_out"], np.float32)
    b_out = np.asarray(inputs["b_out"], np.float32)

    in_maps = make_in_maps(x, w_qkv, b_qkv, w_out)
    nc = build_nc()
    res = run_bass_kernel_spmd(nc, in_maps, core_ids=list(range(8)))
    kernel.last_results = res

    out = np.zeros((B, L, D), np.float32)
    for core in range(8):
        out[core // 4] += res.results[core]["out"]
    out += b_out[None, None, :]
    return out


kernel.last_results = None


# revision 14
# speedup vs baseline: 1.7066x; 1.6791x over previous
"""Trainium2 Bass kernel: multi-head self-attention (B=2, L=2048, D=1024, H=16).

Sharding: 8 NeuronCores = 2 batches x 4 head-groups (4 heads per core).
Each core computes, for its batch and its 4 heads:
  qkv projection -> full attention -> partial out-projection (its heads'
  contribution to out @ w_out).  The host sums the 4 head-group partials per
  batch and adds b_out.

Device dataflow (all layouts chosen so that no on-chip transpose is needed):
  - host passes x^T  [D, L]  (d-major), so d is on SBUF partitions.
  - qkT  = w_qk.T @ x^T      -> [c=512, L]   (Q^T / K^T per head, dk on partitions)
  - V    = x^T.T  @ w_v_aug  -> [L, 260]     (k-major V, plus a ones column per
                                              head that yields the softmax
                                              denominator for free)
  - S^T  = (K^T)ᵀ@ Q^T       -> [k, q] tiles (per head; 2 heads packed in the
                                              128-partition dim, contraction 64)
  - P^T  = exp(S^T)          (no max-subtraction: |scores| <= ~10 in f32, safe)
  - O^T_aug = V_augᵀ... psum += V_aug[k,65].T-contract -> [65, q]
              rows 0-63 = unnormalized head output (dv-major), row 64 = sum_k P
  - normalize: O^T = O^T_aug[0:64] * bcast(1/row64)
  - y    = O^T_cat.T @ w_out_local -> [L, 1024] partial, DMA'd out.

Matmuls run as float32r (TF32-like single-pass, full PE rate for moving dim
>= 256) via bitcast of f32 data; accumulation is f32 in PSUM.
"""

import sys

if "/opt/trn_rl_repo" not in sys.path:
    sys.path.insert(0, "/opt/trn_rl_repo")

import ml_dtypes
import numpy as np

import concourse.bass as bass
import concourse.tile as tile
from concourse import mybir
from concourse.bass_utils import run_bass_kernel_spmd
from concourse.vector_clock import ScopedClock

B, L, D, H, DK = 2, 2048, 1024, 16, 64
HG = 4  # heads per core
F32 = mybir.dt.float32
F32R = mybir.dt.float32r
BF16 = mybir.dt.bfloat16
QC = 512  # l/q chunk width
NQ = L // QC  # 4 chunks
LT = L // 128  # 16 l tiles
KO = D // 128  # 8 contraction subtiles
CV = HG * (DK + 1)  # 260: v columns + per-head ones column

_PATCHED = False


def _patch_tile_drain():
    """This container's walrus rejects >1 sem wait on a ctrl instruction
    (setupSyncWait: 'Too many sync wait commands').  Tile's end-of-kernel
    drain accumulates one wait per outstanding semaphore; split the extras
    onto dedicated nops (same semantics: SP observes every sem before the
    final all-engine barrier)."""
    global _PATCHED
    if _PATCHED:
        return

    def _drain_and_barrier(self, tick_clock, wait_clock):
        nc = self.nc
        drain_inst = nc.sync.drain()
        wait_clock.add_sem_waits(
            drain_inst.ins, ScopedClock({None: tick_clock.global_clock})
        )
        si = drain_inst.ins.sync_info
        waits = list(si.on_wait or []) if si is not None else []
        if len(waits) > 1:
            si.on_wait = waits[:1]
            for w in waits[1:]:
                nop = nc.sync.nop()
                nsi = nop.ins.sync_info
                if nsi is None:
                    nop.ins.sync_info = mybir.SyncInfo(on_wait=[w], on_update=[])
                else:
                    nsi.on_wait = [w]
        nc.all_engine_barrier()
        popped = nc._tile_sem_poison_stack.pop()
        assert popped is self._sem_poison
        nc.clear_and_free_semaphores(list(self.sems.allocated().values()))
        nc.all_engine_barrier()

    tile.TileContext._drain_and_barrier = _drain_and_barrier
    _PATCHED = True


def _split_excess_waits(nc, max_waits=1):
    """This toolchain's walrus/ISA config allows only one sem wait per
    instruction, but Tile's wait assignment can attach several.  Hoist the
    extras onto same-engine nops immediately before the instruction (AND
    semantics preserved: the engine blocks on each in program order)."""
    for f in nc.m.functions:
        for blk in f.blocks:
            insts = list(blk.instructions)
            out = []
            changed = False
            for inst in insts:
                si = inst.sync_info
                waits = list(si.on_wait) if (si is not None and si.on_wait) else []
                if len(waits) > max_waits:
                    changed = True
                    for w in waits[:-max_waits]:
                        nop = mybir.InstNoOp(
                            name=f"I-wsplit-{nc.next_id()}",
                            engine=inst.engine,
                            ins=[],
                            outs=[],
                            sync_info=mybir.SyncInfo(on_wait=[w], on_update=[]),
                        )
                        nc.register_instruction(nop, overwrite=True)
                        out.append(nop)
                    si.on_wait = waits[-max_waits:]
                out.append(inst)
            if changed:
                blk.instructions = out


def build_nc():
    _patch_tile_drain()
    nc = bass.Bass()
    xT = nc.declare_dram_parameter("xT", [D, L], BF16, isOutput=False)
    wqk = nc.declare_dram_parameter("wqk", [D, 512], BF16, isOutput=False)
    bqk = nc.declare_dram_parameter("bqk", [512], F32, isOutput=False)
    wv = nc.declare_dram_parameter("wv", [D, CV], BF16, isOutput=False)
    bv = nc.declare_dram_parameter("bv", [CV], BF16, isOutput=False)
    wout = nc.declare_dram_parameter("wout", [2 * 128, 1024], BF16, isOutput=False)
    ones = nc.declare_dram_parameter("ones", [1, L], BF16, isOutput=False)
    y = nc.declare_dram_parameter("out", [L, D], F32, isOutput=True)

    Ident = mybir.ActivationFunctionType.Identity
    Exp = mybir.ActivationFunctionType.Exp

    with tile.TileContext(nc) as tc:
        with tc.tile_pool(name="per", bufs=1) as per:
            wqk_sb = per.tile([128, KO, 512], BF16)
            wv_sb = per.tile([128, KO, CV], BF16)
            wout_sb = per.tile([128, 2, 1024], BF16)
            bqk_sb = per.tile([128, 4], F32)
            bv_sb = per.tile([1, CV], BF16)
            ones_sb = per.tile([1, L], BF16)
            qkT_sb = per.tile([128, 4, L], BF16)
            v_sb = per.tile([128, LT, CV], BF16)
            oT_sb = per.tile([128, 2, L], BF16)

            for o in range(KO):
                nc.sync.dma_start(out=wqk_sb[:, o, :], in_=wqk[o * 128 : (o + 1) * 128, :])
                nc.sync.dma_start(out=wv_sb[:, o, :], in_=wv[o * 128 : (o + 1) * 128, :])
            nc.sync.dma_start(out=wout_sb[:, 0, :], in_=wout[0:128, :])
            nc.sync.dma_start(out=wout_sb[:, 1, :], in_=wout[128:256, :])
            nc.sync.dma_start(out=bqk_sb[:], in_=bqk.rearrange("(s p) -> p s", p=128))
            nc.sync.dma_start(out=bv_sb[:], in_=bv[None, :])
            nc.sync.dma_start(out=ones_sb[:], in_=ones[:])

            with (
                tc.tile_pool(name="xtp", bufs=1) as xtp,
                tc.tile_pool(name="psB", bufs=1, space="PSUM") as psB,
                tc.tile_pool(name="psC", bufs=2, space="PSUM") as psC,
            ):
                xT_sb = xtp.tile([128, KO, L], BF16)
                for o in range(KO):
                    nc.sync.dma_start(
                        out=xT_sb[:, o, :], in_=xT[o * 128 : (o + 1) * 128, :]
                    )
                # qkT = w_qk.T @ x^T ; bias folded into the PSUM->SBUF copyback.
                # o (the d contraction) is the middle loop so the first matmuls
                # only need the first x^T chunk - PE starts while x^T streams in.
                for s in range(4):
                    pss = [
                        psB.tile([128, QC], F32, tag=f"psqk{n}", name=f"psqk{n}")
                        for n in range(NQ)
                    ]
                    for o in range(KO):
                        for n in range(NQ):
                            nc.tensor.matmul(
                                pss[n][:],
                                wqk_sb[:, o, s * 128 : (s + 1) * 128],
                                xT_sb[:, o, n * QC : (n + 1) * QC],
                                start=(o == 0),
                                stop=(o == KO - 1),
                            )
                    for n in range(NQ):
                        nc.scalar.activation(
                            qkT_sb[:, s, n * QC : (n + 1) * QC],
                            pss[n][:],
                            Ident,
                            bias=bqk_sb[:, s : s + 1],
                            scale=1.0,
                        )
                # V_aug = x @ w_v_aug (+ K=1 matmul adding bias and ones col)
                for lt in range(LT):
                    ps = psC.tile([128, CV], F32, tag="psv")
                    for o in range(KO):
                        nc.tensor.matmul(
                            ps[:],
                            xT_sb[:, o, lt * 128 : (lt + 1) * 128],
                            wv_sb[:, o, :],
                            start=(o == 0),
                            stop=False,
                        )
                    nc.tensor.matmul(
                        ps[:],
                        ones_sb[0:1, 0:128],
                        bv_sb[0:1, :],
                        start=False,
                        stop=True,
                    )
                    nc.vector.tensor_copy(out=v_sb[:, lt, :], in_=ps[:])

            # attention: per head pair (even head on partitions 0-63, odd on
            # 64-127 -> two row-group-packed K=64 matmuls run concurrently)
            with (
                tc.tile_pool(name="pt", bufs=3) as ptp,
                tc.tile_pool(name="rcp", bufs=2) as rcp,
                tc.tile_pool(name="rdp", bufs=2, space="DRAM") as rdp,
                tc.tile_pool(name="psST", bufs=2, space="PSUM") as psST,
                tc.tile_pool(name="psOT", bufs=2, space="PSUM") as psOT,
            ):
                def st_pair(sq, sk, qc, kt):
                    """S^T for both heads of the pair: two K=64 matmuls packed
                    into row groups 0-63 / 64-127 (concurrent on the PE)."""
                    pse = psST.tile([128, QC], F32, tag="ste")
                    pso = psST.tile([128, QC], F32, tag="sto")
                    nc.tensor.matmul(
                        pse[:],
                        qkT_sb[0:64, sk, kt * 128 : (kt + 1) * 128],
                        qkT_sb[0:64, sq, qc * QC : (qc + 1) * QC],
                        start=True,
                        stop=True,
                    )
                    nc.tensor.matmul(
                        pso[:],
                        qkT_sb[64:128, sk, kt * 128 : (kt + 1) * 128],
                        qkT_sb[64:128, sq, qc * QC : (qc + 1) * QC],
                        start=True,
                        stop=True,
                    )
                    return pse, pso

                for p2 in range(2):
                    sq = 2 * p2  # q slot in qkT_sb
                    sk = 2 * p2 + 1  # k slot
                    for qc in range(NQ):
                        po_e = psOT.tile([65, QC], F32, tag="ote")
                        po_o = psOT.tile([65, QC], F32, tag="oto")
                        # software pipeline: S^T runs one kt ahead of exp and
                        # the O^T accumulation so the PE never waits on ACT.
                        sts = st_pair(sq, sk, qc, 0)
                        for kt in range(LT):
                            pse, pso = sts
                            pte = ptp.tile([128, QC], BF16, tag="pte")
                            pto = ptp.tile([128, QC], BF16, tag="pto")
                            nc.scalar.activation(pte[:], pse[:], Exp)
                            nc.scalar.activation(pto[:], pso[:], Exp)
                            if kt + 1 < LT:
                                sts = st_pair(sq, sk, qc, kt + 1)
                            nc.tensor.matmul(
                                po_e[:],
                                v_sb[:, kt, (2 * p2) * 65 : (2 * p2) * 65 + 65],
                                pte[:],
                                start=(kt == 0),
                                stop=(kt == LT - 1),
                            )
                            nc.tensor.matmul(
                                po_o[:],
                                v_sb[:, kt, (2 * p2 + 1) * 65 : (2 * p2 + 1) * 65 + 65],
                                pto[:],
                                start=(kt == 0),
                                stop=(kt == LT - 1),
                            )
                        for he, po in ((0, po_e), (1, po_o)):
                            # 1/rowsum: bounce the PSUM row through DRAM into a
                            # [128,4] layout so the reciprocal uses all DVE
                            # lanes (a [1,512] reciprocal is ~3.3us, 1 lane),
                            # then bounce back broadcast to 64 partitions.
                            rc = rcp.tile([1, QC], F32, tag="rc")
                            rb = rcp.tile([64, QC], F32, tag="rb")
                            rp = rcp.tile([128, QC // 128], F32, tag="rp")
                            nc.vector.tensor_copy(out=rc[:], in_=po[64:65, :])
                            rd = rdp.tile([1, QC], F32, tag="rd")
                            rd2 = rdp.tile([1, QC], F32, tag="rd2")
                            nc.sync.dma_start(out=rd[:], in_=rc[:])
                            nc.sync.dma_start(
                                out=rp[:],
                                in_=rd[0, :].rearrange("(p f) -> p f", p=128),
                            )
                            nc.vector.reciprocal(out=rp[:], in_=rp[:])
                            nc.sync.dma_start(
                                out=rd2[0, :].rearrange("(p f) -> p f", p=128),
                                in_=rp[:],
                            )
                            nc.sync.dma_start(
                                out=rb[:], in_=rd2[0:1, :].to_broadcast([64, QC])
                            )
                            nc.vector.tensor_mul(
                                out=oT_sb[he * 64 : (he + 1) * 64, p2, qc * QC : (qc + 1) * QC],
                                in0=po[0:64, :],
                                in1=rb[:],
                            )

            # partial out-projection: y = O^T_cat.T @ w_out_local
            with (
                tc.tile_pool(name="ysb", bufs=3) as ysb,
                tc.tile_pool(name="psE", bufs=2, space="PSUM") as psE,
            ):
                for lt in range(LT):
                    for n2 in range(2):
                        ps = psE.tile([128, QC], F32, tag="psy")
                        nc.tensor.matmul(
                            ps[:],
                            oT_sb[:, 0, lt * 128 : (lt + 1) * 128],
                            wout_sb[:, 0, n2 * QC : (n2 + 1) * QC],
                            start=True,
                            stop=False,
                        )
                        nc.tensor.matmul(
                            ps[:],
                            oT_sb[:, 1, lt * 128 : (lt + 1) * 128],
                            wout_sb[:, 1, n2 * QC : (n2 + 1) * QC],
                            start=False,
                            stop=True,
                        )
                        yt = ysb.tile([128, QC], F32, tag="yt")
                        nc.vector.tensor_copy(out=yt[:], in_=ps[:])
                        nc.sync.dma_start(
                            out=y[lt * 128 : (lt + 1) * 128, n2 * QC : (n2 + 1) * QC],
                            in_=yt[:],
                        )
    _split_excess_waits(nc)
    return nc


def make_in_maps(x, w_qkv, b_qkv, w_out):
    """Per-core input shards.  Core i: batch i//4, head group i%4 (4 heads).

    w_qk column order per core: slots of 128 = (pair0 q | pair0 k | pair1 q |
    pair1 k), each slot = [even head (64) | odd head (64)].  The 1/sqrt(dk)
    scale is folded into the q columns (and q bias entries).
    """
    in_maps = []
    for core in range(8):
        b, g = divmod(core, 4)
        heads = [4 * g + j for j in range(HG)]
        xT = np.ascontiguousarray(x[b].T)
        cols, bias = [], []
        for pair in range(2):
            for qk in range(2):
                for j in range(2):
                    h = heads[2 * pair + j]
                    base = h * 3 * DK + qk * DK
                    c = w_qkv[:, base : base + DK]
                    bb = b_qkv[base : base + DK]
                    if qk == 0:
                        c = c * (1.0 / np.sqrt(DK))
                        bb = bb * (1.0 / np.sqrt(DK))
                    cols.append(c)
                    bias.append(bb)
        wqk = np.ascontiguousarray(np.concatenate(cols, axis=1), dtype=np.float32)
        bqk = np.concatenate(bias).astype(np.float32)
        wv = np.zeros((D, CV), np.float32)
        bv = np.zeros((CV,), np.float32)
        for j, h in enumerate(heads):
            base = h * 3 * DK + 2 * DK
            wv[:, 65 * j : 65 * j + 64] = w_qkv[:, base : base + DK]
            bv[65 * j : 65 * j + 64] = b_qkv[base : base + DK]
            bv[65 * j + 64] = 1.0
        wo = np.ascontiguousarray(w_out[g * 256 : (g + 1) * 256, :], dtype=np.float32)
        bf = ml_dtypes.bfloat16
        in_maps.append(
            {
                "xT": xT.astype(bf),
                "wqk": wqk.astype(bf),
                "bqk": bqk,
                "wv": wv.astype(bf),
                "bv": bv.astype(bf),
                "wout": wo.astype(bf),
                "ones": np.ones((1, L), bf),
            }
        )
    return in_maps


def kernel(**inputs):
    x = np.asarray(inputs["x"], np.float32)
    w_qkv = np.asarray(inputs["w_qkv"], np.float32)
    b_qkv = np.asarray(inputs["b_qkv"], np.float32)
    w_out = np.asarray(inputs["w_out"], np.float32)
    b_out = np.asarray(inputs["b_out"], np.float32)

    in_maps = make_in_maps(x, w_qkv, b_qkv, w_out)
    nc = build_nc()
    res = run_bass_kernel_spmd(nc, in_maps, core_ids=list(range(8)))
    kernel.last_results = res

    out = np.zeros((B, L, D), np.float32)
    for core in range(8):
        out[core // 4] += res.results[core]["out"]
    out += b_out[None, None, :]
    return out


kernel.last_results = None


# revision 15
# speedup vs baseline: 1.8887x; 1.1067x over previous
"""Trainium2 Bass kernel: multi-head self-attention (B=2, L=2048, D=1024, H=16).

Sharding: 8 NeuronCores = 2 batches x 4 head-groups (4 heads per core).
Each core computes, for its batch and its 4 heads:
  qkv projection -> full attention -> partial out-projection (its heads'
  contribution to out @ w_out).  The host sums the 4 head-group partials per
  batch and adds b_out.

Device dataflow (all layouts chosen so that no on-chip transpose is needed):
  - host passes x^T  [D, L]  (d-major), so d is on SBUF partitions.
  - qkT  = w_qk.T @ x^T      -> [c=512, L]   (Q^T / K^T per head, dk on partitions)
  - V    = x^T.T  @ w_v_aug  -> [L, 260]     (k-major V, plus a ones column per
                                              head that yields the softmax
                                              denominator for free)
  - S^T  = (K^T)ᵀ@ Q^T       -> [k, q] tiles (per head; 2 heads packed in the
                                              128-partition dim, contraction 64)
  - P^T  = exp(S^T)          (no max-subtraction: |scores| <= ~10 in f32, safe)
  - O^T_aug = V_augᵀ... psum += V_aug[k,65].T-contract -> [65, q]
              rows 0-63 = unnormalized head output (dv-major), row 64 = sum_k P
  - normalize: O^T = O^T_aug[0:64] * bcast(1/row64)
  - y    = O^T_cat.T @ w_out_local -> [L, 1024] partial, DMA'd out.

Matmuls run as float32r (TF32-like single-pass, full PE rate for moving dim
>= 256) via bitcast of f32 data; accumulation is f32 in PSUM.
"""

import sys

if "/opt/trn_rl_repo" not in sys.path:
    sys.path.insert(0, "/opt/trn_rl_repo")

import ml_dtypes
import numpy as np

import concourse.bass as bass
import concourse.tile as tile
from concourse import mybir
from concourse.bass_utils import run_bass_kernel_spmd
from concourse.vector_clock import ScopedClock

B, L, D, H, DK = 2, 2048, 1024, 16, 64
HG = 4  # heads per core
F32 = mybir.dt.float32
F32R = mybir.dt.float32r
BF16 = mybir.dt.bfloat16
QC = 512  # l/q chunk width
NQ = L // QC  # 4 chunks
LT = L // 128  # 16 l tiles
KO = D // 128  # 8 contraction subtiles
CV = HG * (DK + 1)  # 260: v columns + per-head ones column

_PATCHED = False


def _patch_tile_drain():
    """This container's walrus rejects >1 sem wait on a ctrl instruction
    (setupSyncWait: 'Too many sync wait commands').  Tile's end-of-kernel
    drain accumulates one wait per outstanding semaphore; split the extras
    onto dedicated nops (same semantics: SP observes every sem before the
    final all-engine barrier)."""
    global _PATCHED
    if _PATCHED:
        return

    def _drain_and_barrier(self, tick_clock, wait_clock):
        nc = self.nc
        drain_inst = nc.sync.drain()
        wait_clock.add_sem_waits(
            drain_inst.ins, ScopedClock({None: tick_clock.global_clock})
        )
        si = drain_inst.ins.sync_info
        waits = list(si.on_wait or []) if si is not None else []
        if len(waits) > 1:
            si.on_wait = waits[:1]
            for w in waits[1:]:
                nop = nc.sync.nop()
                nsi = nop.ins.sync_info
                if nsi is None:
                    nop.ins.sync_info = mybir.SyncInfo(on_wait=[w], on_update=[])
                else:
                    nsi.on_wait = [w]
        nc.all_engine_barrier()
        popped = nc._tile_sem_poison_stack.pop()
        assert popped is self._sem_poison
        nc.clear_and_free_semaphores(list(self.sems.allocated().values()))
        nc.all_engine_barrier()

    tile.TileContext._drain_and_barrier = _drain_and_barrier
    _PATCHED = True


def _split_excess_waits(nc, max_waits=1):
    """This toolchain's walrus/ISA config allows only one sem wait per
    instruction, but Tile's wait assignment can attach several.  Hoist the
    extras onto same-engine nops immediately before the instruction (AND
    semantics preserved: the engine blocks on each in program order)."""
    for f in nc.m.functions:
        for blk in f.blocks:
            insts = list(blk.instructions)
            out = []
            changed = False
            for inst in insts:
                si = inst.sync_info
                waits = list(si.on_wait) if (si is not None and si.on_wait) else []
                if len(waits) > max_waits:
                    changed = True
                    for w in waits[:-max_waits]:
                        nop = mybir.InstNoOp(
                            name=f"I-wsplit-{nc.next_id()}",
                            engine=inst.engine,
                            ins=[],
                            outs=[],
                            sync_info=mybir.SyncInfo(on_wait=[w], on_update=[]),
                        )
                        nc.register_instruction(nop, overwrite=True)
                        out.append(nop)
                    si.on_wait = waits[-max_waits:]
                out.append(inst)
            if changed:
                blk.instructions = out


def build_nc():
    _patch_tile_drain()
    nc = bass.Bass()
    xT = nc.declare_dram_parameter("xT", [D, L], BF16, isOutput=False)
    wqk = nc.declare_dram_parameter("wqk", [D, 512], BF16, isOutput=False)
    bqk = nc.declare_dram_parameter("bqk", [512], F32, isOutput=False)
    wv = nc.declare_dram_parameter("wv", [D, CV], BF16, isOutput=False)
    bv = nc.declare_dram_parameter("bv", [CV], BF16, isOutput=False)
    wout = nc.declare_dram_parameter("wout", [2 * 128, 1024], BF16, isOutput=False)
    ones = nc.declare_dram_parameter("ones", [1, L], BF16, isOutput=False)
    y = nc.declare_dram_parameter("out", [L, D], F32, isOutput=True)

    Ident = mybir.ActivationFunctionType.Identity
    Exp = mybir.ActivationFunctionType.Exp

    with tile.TileContext(nc) as tc:
        with tc.tile_pool(name="per", bufs=1) as per:
            wqk_sb = per.tile([128, KO, 512], BF16)
            wv_sb = per.tile([128, KO, CV], BF16)
            wout_sb = per.tile([128, 2, 1024], BF16)
            bqk_sb = per.tile([128, 4], F32)
            bv_sb = per.tile([1, CV], BF16)
            ones_sb = per.tile([1, L], BF16)
            qkT_sb = per.tile([128, 4, L], BF16)
            v_sb = per.tile([128, LT, CV], BF16)
            oT_sb = per.tile([128, 2, L], BF16)

            for o in range(KO):
                nc.sync.dma_start(out=wqk_sb[:, o, :], in_=wqk[o * 128 : (o + 1) * 128, :])
                nc.sync.dma_start(out=wv_sb[:, o, :], in_=wv[o * 128 : (o + 1) * 128, :])
            nc.sync.dma_start(out=wout_sb[:, 0, :], in_=wout[0:128, :])
            nc.sync.dma_start(out=wout_sb[:, 1, :], in_=wout[128:256, :])
            nc.sync.dma_start(out=bqk_sb[:], in_=bqk.rearrange("(s p) -> p s", p=128))
            nc.sync.dma_start(out=bv_sb[:], in_=bv[None, :])
            nc.sync.dma_start(out=ones_sb[:], in_=ones[:])

            with (
                tc.tile_pool(name="xtp", bufs=1) as xtp,
                tc.tile_pool(name="psB", bufs=1, space="PSUM") as psB,
                tc.tile_pool(name="psC", bufs=2, space="PSUM") as psC,
            ):
                xT_sb = xtp.tile([128, KO, L], BF16)
                for o in range(KO):
                    nc.sync.dma_start(
                        out=xT_sb[:, o, :], in_=xT[o * 128 : (o + 1) * 128, :]
                    )
                # qkT = w_qk.T @ x^T ; bias folded into the PSUM->SBUF copyback.
                # o (the d contraction) is the middle loop so the first matmuls
                # only need the first x^T chunk - PE starts while x^T streams in.
                for s in range(4):
                    pss = [
                        psB.tile([128, QC], F32, tag=f"psqk{n}", name=f"psqk{n}")
                        for n in range(NQ)
                    ]
                    for o in range(KO):
                        for n in range(NQ):
                            nc.tensor.matmul(
                                pss[n][:],
                                wqk_sb[:, o, s * 128 : (s + 1) * 128],
                                xT_sb[:, o, n * QC : (n + 1) * QC],
                                start=(o == 0),
                                stop=(o == KO - 1),
                            )
                    for n in range(NQ):
                        nc.scalar.activation(
                            qkT_sb[:, s, n * QC : (n + 1) * QC],
                            pss[n][:],
                            Ident,
                            bias=bqk_sb[:, s : s + 1],
                            scale=1.0,
                        )
                # V_aug = x @ w_v_aug (+ K=1 matmul adding bias and ones col)
                for lt in range(LT):
                    ps = psC.tile([128, CV], F32, tag="psv")
                    for o in range(KO):
                        nc.tensor.matmul(
                            ps[:],
                            xT_sb[:, o, lt * 128 : (lt + 1) * 128],
                            wv_sb[:, o, :],
                            start=(o == 0),
                            stop=False,
                        )
                    nc.tensor.matmul(
                        ps[:],
                        ones_sb[0:1, 0:128],
                        bv_sb[0:1, :],
                        start=False,
                        stop=True,
                    )
                    nc.vector.tensor_copy(out=v_sb[:, lt, :], in_=ps[:])

            # attention: per head pair (even head on partitions 0-63, odd on
            # 64-127 -> two row-group-packed K=64 matmuls run concurrently)
            with (
                tc.tile_pool(name="pt", bufs=3) as ptp,
                tc.tile_pool(name="rcp", bufs=2) as rcp,
                tc.tile_pool(name="rdp", bufs=2, space="DRAM") as rdp,
                tc.tile_pool(name="psST", bufs=2, space="PSUM") as psST,
                tc.tile_pool(name="psOT", bufs=2, space="PSUM") as psOT,
            ):
                def st_pair(sq, sk, qc, kt):
                    """S^T for both heads of the pair: two K=64 matmuls packed
                    into row groups 0-63 / 64-127 (concurrent on the PE),
                    writing the two halves of one 2-bank PSUM tile so a single
                    wide ACTIVATE can exp both (amortizes ACT's ~352-cycle
                    fixed overhead per instruction)."""
                    ps2 = psST.tile([128, 2, QC], F32, tag="st2")
                    nc.tensor.matmul(
                        ps2[:, 0, :],
                        qkT_sb[0:64, sk, kt * 128 : (kt + 1) * 128],
                        qkT_sb[0:64, sq, qc * QC : (qc + 1) * QC],
                        start=True,
                        stop=True,
                    )
                    nc.tensor.matmul(
                        ps2[:, 1, :],
                        qkT_sb[64:128, sk, kt * 128 : (kt + 1) * 128],
                        qkT_sb[64:128, sq, qc * QC : (qc + 1) * QC],
                        start=True,
                        stop=True,
                    )
                    return ps2

                for p2 in range(2):
                    sq = 2 * p2  # q slot in qkT_sb
                    sk = 2 * p2 + 1  # k slot
                    for qc in range(NQ):
                        po_e = psOT.tile([65, QC], F32, tag="ote")
                        po_o = psOT.tile([65, QC], F32, tag="oto")
                        # software pipeline: S^T runs one kt ahead of exp and
                        # the O^T accumulation so the PE never waits on ACT.
                        sts = st_pair(sq, sk, qc, 0)
                        for kt in range(LT):
                            ps2 = sts
                            pt2 = ptp.tile([128, 2, QC], BF16, tag="pt2")
                            nc.scalar.activation(pt2[:], ps2[:], Exp)
                            if kt + 1 < LT:
                                sts = st_pair(sq, sk, qc, kt + 1)
                            nc.tensor.matmul(
                                po_e[:],
                                v_sb[:, kt, (2 * p2) * 65 : (2 * p2) * 65 + 65],
                                pt2[:, 0, :],
                                start=(kt == 0),
                                stop=(kt == LT - 1),
                            )
                            nc.tensor.matmul(
                                po_o[:],
                                v_sb[:, kt, (2 * p2 + 1) * 65 : (2 * p2 + 1) * 65 + 65],
                                pt2[:, 1, :],
                                start=(kt == 0),
                                stop=(kt == LT - 1),
                            )
                        for he, po in ((0, po_e), (1, po_o)):
                            # 1/rowsum: bounce the PSUM row through DRAM into a
                            # [128,4] layout so the reciprocal uses all DVE
                            # lanes (a [1,512] reciprocal is ~3.3us, 1 lane),
                            # then bounce back broadcast to 64 partitions.
                            rc = rcp.tile([1, QC], F32, tag="rc")
                            rb = rcp.tile([64, QC], F32, tag="rb")
                            rp = rcp.tile([128, QC // 128], F32, tag="rp")
                            nc.vector.tensor_copy(out=rc[:], in_=po[64:65, :])
                            rd = rdp.tile([1, QC], F32, tag="rd")
                            rd2 = rdp.tile([1, QC], F32, tag="rd2")
                            nc.sync.dma_start(out=rd[:], in_=rc[:])
                            nc.sync.dma_start(
                                out=rp[:],
                                in_=rd[0, :].rearrange("(p f) -> p f", p=128),
                            )
                            nc.vector.reciprocal(out=rp[:], in_=rp[:])
                            nc.sync.dma_start(
                                out=rd2[0, :].rearrange("(p f) -> p f", p=128),
                                in_=rp[:],
                            )
                            nc.sync.dma_start(
                                out=rb[:], in_=rd2[0:1, :].to_broadcast([64, QC])
                            )
                            nc.vector.tensor_mul(
                                out=oT_sb[he * 64 : (he + 1) * 64, p2, qc * QC : (qc + 1) * QC],
                                in0=po[0:64, :],
                                in1=rb[:],
                            )

            # partial out-projection: y = O^T_cat.T @ w_out_local
            with (
                tc.tile_pool(name="ysb", bufs=3) as ysb,
                tc.tile_pool(name="psE", bufs=2, space="PSUM") as psE,
            ):
                for lt in range(LT):
                    for n2 in range(2):
                        ps = psE.tile([128, QC], F32, tag="psy")
                        nc.tensor.matmul(
                            ps[:],
                            oT_sb[:, 0, lt * 128 : (lt + 1) * 128],
                            wout_sb[:, 0, n2 * QC : (n2 + 1) * QC],
                            start=True,
                            stop=False,
                        )
                        nc.tensor.matmul(
                            ps[:],
                            oT_sb[:, 1, lt * 128 : (lt + 1) * 128],
                            wout_sb[:, 1, n2 * QC : (n2 + 1) * QC],
                            start=False,
                            stop=True,
                        )
                        yt = ysb.tile([128, QC], F32, tag="yt")
                        nc.vector.tensor_copy(out=yt[:], in_=ps[:])
                        nc.sync.dma_start(
                            out=y[lt * 128 : (lt + 1) * 128, n2 * QC : (n2 + 1) * QC],
                            in_=yt[:],
                        )
    _split_excess_waits(nc)
    return nc


def make_in_maps(x, w_qkv, b_qkv, w_out):
    """Per-core input shards.  Core i: batch i//4, head group i%4 (4 heads).

    w_qk column order per core: slots of 128 = (pair0 q | pair0 k | pair1 q |
    pair1 k), each slot = [even head (64) | odd head (64)].  The 1/sqrt(dk)
    scale is folded into the q columns (and q bias entries).
    """
    in_maps = []
    for core in range(8):
        b, g = divmod(core, 4)
        heads = [4 * g + j for j in range(HG)]
        xT = np.ascontiguousarray(x[b].T)
        cols, bias = [], []
        for pair in range(2):
            for qk in range(2):
                for j in range(2):
                    h = heads[2 * pair + j]
                    base = h * 3 * DK + qk * DK
                    c = w_qkv[:, base : base + DK]
                    bb = b_qkv[base : base + DK]
                    if qk == 0:
                        c = c * (1.0 / np.sqrt(DK))
                        bb = bb * (1.0 / np.sqrt(DK))
                    cols.append(c)
                    bias.append(bb)
        wqk = np.ascontiguousarray(np.concatenate(cols, axis=1), dtype=np.float32)
        bqk = np.concatenate(bias).astype(np.float32)
        wv = np.zeros((D, CV), np.float32)
        bv = np.zeros((CV,), np.float32)
        for j, h in enumerate(heads):
            base = h * 3 * DK + 2 * DK
            wv[:, 65 * j : 65 * j + 64] = w_qkv[:, base : base + DK]
            bv[65 * j : 65 * j + 64] = b_qkv[base : base + DK]
            bv[65 * j + 64] = 1.0
        wo = np.ascontiguousarray(w_out[g * 256 : (g + 1) * 256, :], dtype=np.float32)
        bf = ml_dtypes.bfloat16
        in_maps.append(
            {
                "xT": xT.astype(bf),
                "wqk": wqk.astype(bf),
                "bqk": bqk,
                "wv": wv.astype(bf),
                "bv": bv.astype(bf),
                "wout": wo.astype(bf),
                "ones": np.ones((1, L), bf),
            }
        )
    return in_maps


def kernel(**inputs):
    x = np.asarray(inputs["x"], np.float32)
    w_qkv = np.asarray(inputs["w_qkv"], np.float32)
    b_qkv = np.asarray(inputs["b_qkv"], np.float32)
    w_out = np.asarray(inputs["w_out"], np.float32)
    b_out = np.asarray(inputs["b_out"], np.float32)

    in_maps = make_in_maps(x, w_qkv, b_qkv, w_out)
    nc = build_nc()
    res = run_bass_kernel_spmd(nc, in_maps, core_ids=list(range(8)))
    kernel.last_results = res

    out = np.zeros((B, L, D), np.float32)
    for core in range(8):
        out[core // 4] += res.results[core]["out"]
    out += b_out[None, None, :]
    return out


kernel.last_results = None
